# revision 15
# baseline (speedup 1.0000x reference)
"""Trainium2 Bass kernel for a 2-layer RGCN (mean aggregation) + sigmoid head.

Math (per reference):
  h1 = relu( sum_r mean_{e:(dst,r)} x[src] @ W1[r] + x @ root1 + b1 )
  h2 = relu( sum_r mean_{e:(dst,r)} h1[src] @ W2[r] + h1 @ root2 + b2 )
  out = sigmoid(h2 @ Ws + bs)

Strategy (8 NeuronCores, dst-sharded, bf16 compute / fp32 PSUM):
  - Aggregate-then-transform per dst block of 128 nodes: segmented mean via
    one-hot matmuls.  All 8 relations of a dst block accumulate into one
    [128, 1024] PSUM tile (columns = rpair*256 + (r&1)*128 + dst_local), so
    each 128-edge slot block needs a single one-hot build + a single matmul
    (two in layer 2, for the two 128-wide halves of H=256).
  - Edges are bucketed by (dst_block, src_half, rpair); src_half splits the
    node table at 32768 so single-row gathers fit signed int16 indices
    (256B/512B rows, half the traffic of the pair-table trick).
  - One-hot S (with 1/cnt folded in) built by DVE/GpSimd tensor_scalar on a
    bf16 iota; transform weights are applied from SBUF-resident bf16 copies.
  - h1 is written in bf16, AllGather'd, and also kept transposed in SBUF
    (via PE transpose) for the layer-2 root term; the scoring head is a
    tensor_tensor_reduce against a broadcast Ws followed by a sigmoid.
"""

import os

import numpy as np

import concourse.bacc as bacc
import concourse.bass as bass
import concourse.mybir as mybir
import concourse.tile as tile
import concourse.bass_utils as bass_utils

F32 = mybir.dt.float32
BF = mybir.dt.bfloat16
I16 = mybir.dt.int16
NPBF = mybir.dt.np(BF)

NC = 8          # cores
HALF = 32768    # node-table split so gather indices fit int16
G = 32          # max gather batch (slot blocks per dma_gather)


# ---------------------------------------------------------------------------
# Host-side scheduling
# ---------------------------------------------------------------------------

def _build_schedule(src, dst, etype, N, R, n_cores):
    """Bucket edges by (dst_block, src_half, relation_pair); build the
    SPMD-common block/call schedule and per-core slot arrays."""
    ND = N // n_cores                  # dst nodes per core
    NDB = (ND + 127) // 128            # dst blocks per core
    NRP = R // 2                       # relation pairs
    NCH = NDB * 2 * NRP                # chunks per core

    seg = dst * R + etype
    cnt = np.bincount(seg, minlength=N * R)
    nv_all = (1.0 / np.maximum(cnt, 1)).astype(np.float32)[seg]

    core_of = dst // ND
    lens = np.zeros((n_cores, NCH), np.int64)
    per_core = []
    for c in range(n_cores):
        m = core_of == c
        s = src[m]
        dl = dst[m] - c * ND
        t = etype[m]
        nv = nv_all[m]
        h = (s >= HALF).astype(np.int64)
        chunk = ((dl >> 7) * 2 + h) * NRP + (t >> 1)
        order = np.argsort(chunk, kind="stable")
        per_core.append((s[order], dl[order], t[order], nv[order], chunk[order]))
        lens[c] = np.bincount(chunk[order], minlength=NCH)

    nb = np.ceil(lens.max(axis=0) / 128.0).astype(np.int64)
    nbv = nb.reshape(NDB, 2, NRP)
    for db in range(NDB):             # every (db, rp) PSUM slice needs >=1 mm
        for rp in range(NRP):
            if nbv[db, 0, rp] + nbv[db, 1, rp] == 0:
                nbv[db, 0, rp] = 1
    nb = nbv.reshape(-1)
    NBLK = int(nb.sum())
    NSLOT = NBLK * 128
    slot_base = np.zeros(NCH, np.int64)
    slot_base[1:] = np.cumsum(nb * 128)[:-1]

    # gather calls: maximal same-half runs of blocks, capped at G
    blk_half = np.repeat((np.arange(NCH) // NRP) % 2, nb)
    calls = []
    b = 0
    while b < NBLK:
        h = blk_half[b]
        w = 1
        while b + w < NBLK and blk_half[b + w] == h and w < G:
            w += 1
        calls.append((b, w, int(h)))
        b += w
    b2c = np.zeros(NBLK, np.int64)     # block -> call index
    b2j = np.zeros(NBLK, np.int64)     # block -> position within call
    for ci, (b0, w, h) in enumerate(calls):
        b2c[b0:b0 + w] = ci
        b2j[b0:b0 + w] = np.arange(w)

    cores = []
    for c in range(n_cores):
        s, dl, t, nv, chunk = per_core[c]
        M = len(s)
        chunk_start = np.zeros(NCH, np.int64)
        chunk_start[1:] = np.cumsum(lens[c])[:-1]
        pos = slot_base[chunk] + (np.arange(M) - chunk_start[chunk])

        gidx = np.zeros(NSLOT, np.int64)
        colid = np.full(NSLOT, -1.0, np.float32)
        nvs = np.zeros(NSLOT, np.float32)
        gidx[pos] = np.where(s >= HALF, s - HALF, s)
        colid[pos] = (t & 1) * 128 + (dl & 127)
        nvs[pos] = nv

        idx16 = np.tile(gidx.astype(np.int16).reshape(NSLOT // 16, 16).T,
                        (8, 1))                                  # [128, NSLOT/16]
        colid_m = colid.reshape(NBLK, 128).T                     # [128, NBLK]
        nv_m = nvs.reshape(NBLK, 128).T
        cores.append(dict(idx16=np.ascontiguousarray(idx16),
                          colid=np.ascontiguousarray(colid_m),
                          nv=np.ascontiguousarray(nv_m)))

    return dict(ND=ND, NDB=NDB, NRP=NRP, nbv=nbv, NBLK=NBLK, NSLOT=NSLOT,
                calls=calls, b2c=b2c, b2j=b2j, cores=cores)


# ---------------------------------------------------------------------------
# Device program
# ---------------------------------------------------------------------------

def _build_program(N, F, H, O, R, n_cores, sched):
    ND, NDB, NRP = sched["ND"], sched["NDB"], sched["NRP"]
    nbv, NBLK, NSLOT = sched["nbv"], sched["NBLK"], sched["NSLOT"]
    calls, b2c, b2j = sched["calls"], sched["b2c"], sched["b2j"]
    NDP = NDB * 128
    SW = 2 * NRP * 128            # one-hot width per dst block (= R*128)

    nc = bacc.Bacc("TRN2", target_bir_lowering=False, debug=False,
                   num_devices=n_cores, num_swdge_queues=4)

    # ---- I/O ----
    xbf_d = nc.dram_tensor("xbf", [N, F], BF, kind="ExternalInput")
    xt_d = nc.dram_tensor("xt", [128, NDP], BF, kind="ExternalInput")
    idx_d = nc.dram_tensor("idx16", [128, NSLOT // 16], I16, kind="ExternalInput")
    cid_d = nc.dram_tensor("colid", [128, NBLK], F32, kind="ExternalInput")
    nv_d = nc.dram_tensor("nv", [128, NBLK], F32, kind="ExternalInput")
    w1_d = nc.dram_tensor("w1", [128, R * H], BF, kind="ExternalInput")
    w2_d = nc.dram_tensor("w2", [128, R * 2 * O], BF, kind="ExternalInput")
    r1_d = nc.dram_tensor("r1", [128, H], BF, kind="ExternalInput")
    r2_d = nc.dram_tensor("r2", [128, 2 * O], BF, kind="ExternalInput")
    b1_d = nc.dram_tensor("b1", [1, H], BF, kind="ExternalInput")
    b2_d = nc.dram_tensor("b2", [1, O], BF, kind="ExternalInput")
    io_d = nc.dram_tensor("iota", [128, 256], F32, kind="ExternalInput")
    id_d = nc.dram_tensor("ident", [128, 128], F32, kind="ExternalInput")
    wsb_d = nc.dram_tensor("wsb", [128, O], BF, kind="ExternalInput")
    bsc_d = nc.dram_tensor("bsc", [128, 1], F32, kind="ExternalInput")
    sc_d = nc.dram_tensor("scores", [128, NDB], F32, kind="ExternalOutput")

    eq, mul = mybir.AluOpType.is_equal, mybir.AluOpType.mult

    with tile.TileContext(nc) as tc:
        with (
            tc.tile_pool(name="const", bufs=1) as cp,
            tc.tile_pool(name="dram", bufs=1, space="DRAM") as dramp,
        ):
            def load_const(d, shape, dtype=BF):
                t = cp.tile(shape, dtype, tag=d.name)
                nc.sync.dma_start(t[:], d[:])
                return t

            idx_s = load_const(idx_d, [128, NSLOT // 16], I16)
            cid_s = load_const(cid_d, [128, NBLK], F32)
            nv_s = load_const(nv_d, [128, NBLK], F32)
            w1_s = load_const(w1_d, [128, R * H])
            w2_s = load_const(w2_d, [128, R * 2 * O])
            r1_s = load_const(r1_d, [128, H])
            r2_s = load_const(r2_d, [128, 2 * O])
            b1_s = load_const(b1_d, [1, H])
            b2_s = load_const(b2_d, [1, O])
            io_s = load_const(io_d, [128, 256], F32)
            id_s = load_const(id_d, [128, 128], F32)
            wsb_s = load_const(wsb_d, [128, O])
            bsc_s = load_const(bsc_d, [128, 1], F32)
            xt_s = load_const(xt_d, [128, NDP])
            ones1 = cp.tile([1, 128], BF, tag="ones1")
            nc.vector.memset(ones1[:], 1.0)

            h1t_s = cp.tile([128, 2 * NDP], BF, tag="h1t")   # h1^T, 2 h-slabs
            sc_sb = cp.tile([128, NDB], F32, tag="scsb")

            h1loc = dramp.tile([NDP, H], BF)
            h1full = dramp.tile([N, H], BF)

            def agg_loop(ringp, sp, elem, lo_ap, hi_ap, reg_of, mpb, emit_mms):
                """Shared aggregation driver: walks blocks in schedule order,
                fires batched gathers, builds one-hots, and calls emit_mms
                per block with per-matmul (start, stop) flags.  PSUM zero
                regions are 2KB banks, so start/stop are tracked per REGION
                (reg_of maps rp -> region; mpb = matmuls per block): start
                fires only on the first matmul into a bank, stop on the
                last."""
                tiles = {}

                def get_tile(ci):
                    if ci in tiles:
                        return tiles[ci]
                    b0, w, h = calls[ci]
                    t = ringp.tile([128, G * elem], BF, tag="xr")
                    nc.gpsimd.dma_gather(
                        t[:, : w * elem].rearrange("p (g f) -> p g f", f=elem),
                        lo_ap if h == 0 else hi_ap,
                        idx_s[:, b0 * 8:(b0 + w) * 8],
                        w * 128,
                        w * 128,
                        elem,
                        single_packet=False,
                    )
                    tiles[ci] = t
                    return t

                nreg = max(reg_of) + 1
                b = 0
                for db in range(NDB):
                    tot = [0] * nreg
                    for rp in range(NRP):
                        tot[reg_of[rp]] += int(nbv[db, 0, rp] +
                                               nbv[db, 1, rp]) * mpb
                    k_reg = [0] * nreg
                    slices = yield db          # new psum tile(s) for this db
                    for h in range(2):
                        for rp in range(NRP):
                            for _ in range(int(nbv[db, h, rp])):
                                ci = int(b2c[b])
                                j = int(b2j[b])
                                xr = get_tile(ci)
                                se = sp.tile([128, 256], BF, tag="se")
                                nc.vector.tensor_scalar(
                                    se[:], io_s[:], cid_s[:, b:b + 1],
                                    nv_s[:, b:b + 1], op0=eq, op1=mul)
                                reg = reg_of[rp]
                                flags = []
                                for _m in range(mpb):
                                    flags.append((k_reg[reg] == 0,
                                                  k_reg[reg] == tot[reg] - 1))
                                    k_reg[reg] += 1
                                emit_mms(slices, rp, xr, j, se, flags)
                                b += 1
                    yield None                 # db done -> transform phase

            # =============== LAYER 1 ===============
            with (
                tc.tile_pool(name="ring1", bufs=3) as ringp,
                tc.tile_pool(name="s1", bufs=4) as sp,
                tc.tile_pool(name="at1", bufs=2) as atp,
                tc.tile_pool(name="h1sb", bufs=2) as h1p,
                tc.tile_pool(name="pat1", bufs=2, space="PSUM") as patp,
                tc.tile_pool(name="ph1", bufs=2, space="PSUM") as php,
                tc.tile_pool(name="ptr1", bufs=2, space="PSUM") as ptrp,
            ):
                def emit1(psum_at, rp, xr, j, se, flags):
                    nc.tensor.matmul(psum_at[:, rp * 256:(rp + 1) * 256],
                                     xr[:, j * F:(j + 1) * F], se[:],
                                     start=flags[0][0], stop=flags[0][1])

                drv = agg_loop(ringp, sp, F, xbf_d[0:HALF, :], xbf_d[HALF:N, :],
                               [0, 0, 1, 1], 1, emit1)
                for db in drv:
                    psum_at = patp.tile([128, SW], F32)
                    drv.send(psum_at)
                    # transform phase for dst block db
                    at_sb = atp.tile([128, SW], BF)
                    nc.vector.tensor_copy(at_sb[:], psum_at[:])
                    psum_h = php.tile([128, H], F32)
                    nc.tensor.matmul(psum_h[:], xt_s[:, db * 128:(db + 1) * 128],
                                     r1_s[:], start=True, stop=False)
                    for r in range(R):
                        off = (r >> 1) * 256 + (r & 1) * 128
                        nc.tensor.matmul(psum_h[:], at_sb[:, off:off + 128],
                                         w1_s[:, r * H:(r + 1) * H],
                                         start=False, stop=False)
                    nc.tensor.matmul(psum_h[:], ones1[:], b1_s[:],
                                     start=False, stop=True)
                    h1_sb = h1p.tile([128, H], BF)
                    nc.vector.tensor_scalar_max(h1_sb[:], psum_h[:], 0.0)
                    nc.sync.dma_start(h1loc[db * 128:(db + 1) * 128, :], h1_sb[:])
                    h1_sf = h1p.tile([128, H], F32, tag="h1sf")
                    nc.scalar.activation(h1_sf[:], psum_h[:],
                                         mybir.ActivationFunctionType.Relu)
                    for sl in range(2):
                        pt = ptrp.tile([128, 128], F32, tag="ptr")
                        nc.tensor.transpose(pt[:], h1_sf[:, sl * 128:(sl + 1) * 128],
                                            id_s[:])
                        nc.vector.tensor_copy(
                            h1t_s[:, sl * NDP + db * 128: sl * NDP + (db + 1) * 128],
                            pt[:])

            # =============== ALLGATHER h1 ===============
            nc.gpsimd.collective_compute(
                "AllGather",
                mybir.AluOpType.bypass,
                replica_groups=[list(range(n_cores))],
                ins=[h1loc[:ND, :].opt()],
                outs=[h1full[:].opt()],
            )

            # =============== LAYER 2 ===============
            with (
                tc.tile_pool(name="ring2", bufs=3) as ringp2,
                tc.tile_pool(name="s2", bufs=4) as sp2,
                tc.tile_pool(name="at2", bufs=2) as atp2,
                tc.tile_pool(name="h2sb", bufs=2) as h2p,
                tc.tile_pool(name="hd", bufs=2) as hdp,
                tc.tile_pool(name="pat2", bufs=1, space="PSUM") as patp2,
                tc.tile_pool(name="ph2", bufs=2, space="PSUM") as php2,
            ):
                def emit2(psum_at2, rp, xr, j, se, flags):
                    for sl in range(2):
                        off = rp * 512 + sl * 256
                        nc.tensor.matmul(
                            psum_at2[:, off: off + 256],
                            xr[:, j * H + sl * 128: j * H + sl * 128 + 128],
                            se[:], start=flags[sl][0], stop=flags[sl][1])

                drv2 = agg_loop(ringp2, sp2, H, h1full[0:HALF, :],
                                h1full[HALF:N, :], list(range(NRP)), 2, emit2)
                for db in drv2:
                    psum_at2 = patp2.tile([128, 2 * SW], F32)
                    drv2.send(psum_at2)
                    at_sb2 = atp2.tile([128, 2 * SW], BF)
                    nc.vector.tensor_copy(at_sb2[:], psum_at2[:])
                    psum_h2 = php2.tile([128, O], F32)
                    for sl in range(2):
                        nc.tensor.matmul(
                            psum_h2[:],
                            h1t_s[:, sl * NDP + db * 128: sl * NDP + (db + 1) * 128],
                            r2_s[:, sl * O:(sl + 1) * O],
                            start=(sl == 0), stop=False)
                    for r in range(R):
                        for sl in range(2):
                            off = (r >> 1) * 512 + sl * 256 + (r & 1) * 128
                            nc.tensor.matmul(
                                psum_h2[:], at_sb2[:, off: off + 128],
                                w2_s[:, (r * 2 + sl) * O:(r * 2 + sl + 1) * O],
                                start=False, stop=False)
                    nc.tensor.matmul(psum_h2[:], ones1[:], b2_s[:],
                                     start=False, stop=True)
                    h2_sb = h2p.tile([128, O], BF)
                    nc.vector.tensor_scalar_max(h2_sb[:], psum_h2[:], 0.0)
                    hd = hdp.tile([128, O], F32, tag="hd")
                    nc.vector.tensor_mul(hd[:], h2_sb[:], wsb_s[:])
                    psc = hdp.tile([128, 1], F32, tag="psc")
                    nc.vector.reduce_sum(psc[:], hd[:],
                                         axis=mybir.AxisListType.X)
                    nc.scalar.activation(sc_sb[:, db:db + 1], psc[:],
                                         mybir.ActivationFunctionType.Sigmoid,
                                         bias=bsc_s[:, 0:1])
                nc.sync.dma_start(sc_d[:], sc_sb[:])

    nc.compile()
    return nc


# ---------------------------------------------------------------------------
# Entry point
# ---------------------------------------------------------------------------

def prepare(x, edge_index, edge_type, W1, root1, b1, W2, root2, b2, Ws, bs):
    x = np.ascontiguousarray(np.asarray(x, np.float32))
    ei = np.asarray(edge_index)
    et = np.asarray(edge_type).astype(np.int64)
    src, dst = ei[0].astype(np.int64), ei[1].astype(np.int64)
    W1 = np.asarray(W1, np.float32)
    root1 = np.asarray(root1, np.float32)
    b1 = np.asarray(b1, np.float32)
    W2 = np.asarray(W2, np.float32)
    root2 = np.asarray(root2, np.float32)
    b2 = np.asarray(b2, np.float32)
    Ws = np.asarray(Ws, np.float32)
    bs = np.asarray(bs, np.float32)

    N, F = x.shape
    R, _, H = W1.shape
    O = W2.shape[2]

    sched = _build_schedule(src, dst, et, N, R, NC)
    ND, NDB = sched["ND"], sched["NDB"]
    NDP = NDB * 128

    nc = _build_program(N, F, H, O, R, NC, sched)

    # common (replicated) inputs
    xbf = x.astype(NPBF)
    w1f = np.concatenate([W1[r] for r in range(R)], axis=1)
    w2f = np.concatenate(
        [W2[r][sl * 128:(sl + 1) * 128, :] for r in range(R) for sl in range(2)],
        axis=1)
    r2f = np.concatenate([root2[0:128, :], root2[128:256, :]], axis=1)
    iota = np.tile(np.arange(256, dtype=np.float32), (128, 1))
    ident = np.eye(128, dtype=np.float32)
    wsb = np.tile(Ws.reshape(1, O), (128, 1))
    bsc = np.full((128, 1), float(bs.reshape(-1)[0]), np.float32)

    common = dict(
        xbf=np.ascontiguousarray(xbf),
        w1=np.ascontiguousarray(w1f.astype(NPBF)),
        w2=np.ascontiguousarray(w2f.astype(NPBF)),
        r1=np.ascontiguousarray(root1.astype(NPBF)),
        r2=np.ascontiguousarray(r2f.astype(NPBF)),
        b1=np.ascontiguousarray(b1.reshape(1, H).astype(NPBF)),
        b2=np.ascontiguousarray(b2.reshape(1, O).astype(NPBF)),
        iota=np.ascontiguousarray(iota),
        ident=np.ascontiguousarray(ident),
        wsb=np.ascontiguousarray(wsb.astype(NPBF)),
        bsc=bsc,
    )

    in_maps = []
    for c in range(NC):
        xt = np.zeros((128, NDP), NPBF)
        xt[:, :ND] = x[c * ND:(c + 1) * ND].T.astype(NPBF)
        m = dict(common)
        m.update(
            xt=np.ascontiguousarray(xt),
            idx16=sched["cores"][c]["idx16"],
            colid=np.ascontiguousarray(sched["cores"][c]["colid"]),
            nv=np.ascontiguousarray(sched["cores"][c]["nv"]),
        )
        in_maps.append(m)
    return nc, in_maps, ND


def kernel(x, edge_index, edge_type, W1, root1, b1, W2, root2, b2, Ws, bs):
    nc, in_maps, ND = prepare(x, edge_index, edge_type, W1, root1, b1,
                              W2, root2, b2, Ws, bs)
    trace = bool(int(os.environ.get("K_TRACE", "0")))
    res = bass_utils.run_bass_kernel_spmd(nc, in_maps, core_ids=list(range(NC)),
                                          trace=trace)
    global last_exec_time_ns, last_results, last_nc, last_in_maps
    last_results = res
    last_exec_time_ns = res.exec_time_ns
    last_nc = nc
    last_in_maps = in_maps
    out = np.concatenate(
        [res.results[c]["scores"].T.reshape(-1)[:ND] for c in range(NC)])
    return out.astype(np.float32)


if __name__ == "__main__":
    import reference
    inputs = {k: np.asarray(v) for k, v in reference.setup_inputs().items()}
    got = kernel(**inputs)
    exp = np.asarray(reference.reference(**inputs))
    err = np.abs(got - exp).max()
    rel = np.linalg.norm(got - exp) / np.linalg.norm(exp)
    print(f"max abs err {err:.3e}  rel {rel:.3e}")
